# revision 1
# baseline (speedup 1.0000x reference)
"""ClusteringAffinity (vq_codebook) Trainium2 kernel — 8 NeuronCores, SPMD.

Math: out[:, :1000] = max over 4 centers of exp(-||f_b - w_{c,j}||^2 / sigma);
out[:, 1000] = rw, a variance-style regularizer over all pairwise center
distances. The mc x mc pairwise matrix is never formed: with
  A = sum_i ||w_i||^2, B = sum_i ||w_i||^4, s = sum_i w_i,
  u = sum_i ||w_i||^2 w_i, G = W^T W  (h x h Gram),
  T = (mc^2 - mc)/2,
  S1 = mc*A - ||s||^2,    S2 = mc*B + A^2 + 2||G||_F^2 - 4 u.s,
  mu = S1/T,              rw = S2/T - mu^2.

Sharding (no collectives -- an 8-rank AllReduce costs ~80us on this stack):
  cores 1..7: distance for 144 classes each (576 centers, zero-padded),
  core 0:     Gram + stats -> rw. One SPMD program, branch on partition id.
Compute in bf16 on the TensorEngine (validated: rw rel err 2.5e-5,
distance rel err 3e-3, both far inside tolerance); f_sq in fp32.
"""

import numpy as np
import ml_dtypes
from contextlib import ExitStack

B = 512
H = 512
NCLS = 1000
NCEN = 4
SIGMA = 10.0
MC = 4000
MC_PAD = 4096
KC = MC_PAD // 128          # 32 contraction chunks for the Gram
SHARD_N = 576               # centers per distance core (cores 1..7)
SHARD_C = SHARD_N // NCEN   # 144 classes per distance core
N_CORES = 8
T_PAIRS = (MC * MC - MC) / 2.0
INV_T = 1.0 / T_PAIRS

_CACHE = {}


def _install_tile_patch():
    """walrus on this stack rejects >1 sync-wait on CTRL-class (Drain/NoOp)
    instructions; TileContext's tail drain carries one wait per active proc.
    Emit one SP nop per wait instead."""
    import re
    import concourse.tile as tile
    from bass_rust import ScopedClock, VectorClock

    if getattr(tile.TileContext, "_drain_split_patched", False):
        return

    def _clock_values(vc):
        m = re.search(r"\[([0-9, ]*)\]", repr(vc))
        s = m.group(1).strip()
        return [int(x) for x in s.split(",")] if s else []

    def _patched(self, tick_clock, wait_clock):
        nc = self.nc
        vals = _clock_values(tick_clock.global_clock)
        for i, v in enumerate(vals):
            if v > 0:
                chunk = [0] * len(vals)
                chunk[i] = v
                nop = nc.sync.nop(nofuse=True, hint="tail_wait")
                wait_clock.add_sem_waits(
                    nop.ins, ScopedClock({None: VectorClock(chunk)})
                )
        nc.sync.drain()
        nc.all_engine_barrier()
        assert self.sems is not None
        popped = nc._tile_sem_poison_stack.pop()
        assert popped is self._sem_poison
        nc.clear_and_free_semaphores(list(self.sems.allocated().values()))
        nc.all_engine_barrier()

    tile.TileContext._drain_and_barrier = _patched
    tile.TileContext._drain_split_patched = True


def _install_wait_split_patch():
    """This walrus build accepts at most ONE sync-wait per instruction.
    Rewrite the BIR before compile: hoist excess on_wait entries onto
    same-engine NoOps inserted immediately before the instruction."""
    import json
    import concourse.bass2jax as bass2jax
    import concourse.bass_utils as bass_utils

    if getattr(bass_utils, "_wait_split_patched", False):
        return
    orig = bass_utils.compile_bir_kernel

    # Opcodes with wide sem-update immediate fields; everything else is
    # capped at +1 on this walrus build.
    _WIDE_UPDATE = {"DMACopy", "EventSemaphore", "DMATranspose"}

    def _rewrite(bir_bytes):
        d = json.loads(bir_bytes)
        nid = 0
        changed = False
        for fn in d.get("functions", []):
            for blk in fn.get("blocks", []):
                insts = blk.get("instructions", [])
                new = []
                for inst in insts:
                    si = inst.get("sync_info")
                    waits = (si or {}).get("on_wait") or []
                    if len(waits) > 1:
                        changed = True
                        for w in waits[:-1]:
                            nid += 1
                            new.append({
                                "ins": [],
                                "name": f"WS-{nid}-{inst['name']}",
                                "opcode": "NoOp",
                                "outs": [],
                                "engine": inst["engine"],
                                "sync_info": {"on_update": [], "on_wait": [w]},
                                "text_hint": "wait_split",
                            })
                        si["on_wait"] = [waits[-1]]
                    new.append(inst)
                    ups = (si or {}).get("on_update") or []
                    if (
                        ups
                        and inst.get("opcode") not in _WIDE_UPDATE
                        and any(
                            u.get("update_mode") == "sem-add-imm"
                            and u.get("update_value", 0) > 1
                            for u in ups
                        )
                    ):
                        changed = True
                        keep, hoist = [], []
                        for u in ups:
                            if (
                                u.get("update_mode") == "sem-add-imm"
                                and u.get("update_value", 0) > 1
                            ):
                                hoist.append(u)
                            else:
                                keep.append(u)
                        si["on_update"] = keep
                        # Drain first: an EventSemaphore fires at engine
                        # commit-time, which for PE precedes the PSUM drain —
                        # signalling there would let consumers read stale PSUM.
                        nid += 1
                        new.append({
                            "debug": 0,
                            "ins": [],
                            "is_reset_sema": False,
                            "name": f"DR-{nid}-{inst['name']}",
                            "opcode": "Drain",
                            "outs": [],
                            "engine": inst["engine"],
                            "sync_info": {"on_update": [], "on_wait": []},
                        })
                        for u in hoist:
                            nid += 1
                            new.append({
                                "debug": 0,
                                "ins": [],
                                "name": f"US-{nid}-{inst['name']}",
                                "opcode": "EventSemaphore",
                                "outs": [],
                                "engine": inst["engine"],
                                "sync_info": {"on_update": [u], "on_wait": []},
                            })
                blk["instructions"] = new
        if not changed:
            return bir_bytes
        return json.dumps(d).encode()

    def patched(bir_json, tmpdir, neff_name="file.neff"):
        return orig(_rewrite(bir_json), tmpdir, neff_name=neff_name)

    bass_utils.compile_bir_kernel = patched
    bass2jax.compile_bir_kernel = patched
    bass_utils._wait_split_patched = True


def _build():
    import concourse.bass as bass
    import concourse.tile as tile
    from concourse import mybir

    _install_tile_patch()
    _install_wait_split_patch()

    dt = mybir.dt
    f32 = dt.float32
    bf16 = dt.bfloat16
    Alu = mybir.AluOpType
    Act = mybir.ActivationFunctionType
    AX = mybir.AxisListType

    nc = bass.Bass()
    W0 = nc.dram_tensor("W0", [MC_PAD, H], bf16, kind="ExternalInput")
    fT = nc.dram_tensor("fT", [H, B], bf16, kind="ExternalInput")
    fR = nc.dram_tensor("fR", [B, H], f32, kind="ExternalInput")
    WT = nc.dram_tensor("WT", [H, SHARD_N], bf16, kind="ExternalInput")
    dist = nc.dram_tensor("dist", [B, SHARD_C], f32, kind="ExternalOutput")
    rw_out = nc.dram_tensor("rw", [1, 1], f32, kind="ExternalOutput")

    with tile.TileContext(nc) as tc, ExitStack() as ctx:
        sb = ctx.enter_context(tc.tile_pool(name="sb", bufs=1))
        wpool = ctx.enter_context(tc.tile_pool(name="wpool", bufs=1))
        spool = ctx.enter_context(tc.tile_pool(name="spool", bufs=2))
        stats_p = ctx.enter_context(tc.tile_pool(name="stats", bufs=KC))
        cols = ctx.enter_context(tc.tile_pool(name="cols", bufs=16))
        dpool = ctx.enter_context(tc.tile_pool(name="dpool", bufs=2))
        psum = ctx.enter_context(tc.tile_pool(name="psum", bufs=1, space="PSUM"))

        # Shared loads issued before the branch so every core's DMA starts as
        # early as possible (each engine's stream is blocked ~10us by table
        # loads; first kernel DMA issues right after).  Cores that don't use
        # a tensor just waste overlapped DMA bandwidth.
        w_b = wpool.tile([128, KC, H], bf16, tag="w0b")
        fT_b = wpool.tile([128, 4, B], bf16, tag="ftb")      # [h128, hc, b]
        WT_b = wpool.tile([128, 4, SHARD_N], bf16, tag="wtb")
        fR_b = wpool.tile([128, 4, H], f32, tag="frb")
        W0_r = W0.rearrange("(kc p) h -> p kc h", p=128)
        nc.sync.dma_start(WT_b[:], WT.rearrange("(kc p) n -> p kc n", p=128))
        nc.sync.dma_start(fT_b[:], fT.rearrange("(kc p) b -> p kc b", p=128))

        pid = nc.partition_id()
        with tc.If(pid > 0) as cmp:
            # ----- distance branch (cores 1..7) -----
            nc.sync.dma_start(fR_b[:], fR.rearrange("(bc p) h -> p bc h", p=128))
            ones_row = sb.tile([1, 128], bf16, tag="ones_row")
            nc.vector.memset(ones_row[:], 1.0)
            neg_half = sb.tile([128, 1], bf16, tag="neg_half")
            nc.vector.memset(neg_half[:], -0.5)

            # -0.5*||w||^2 as a [1, SHARD_N] row: ones-matmul over squared W^T.
            # Shares the "d" tag's two rotating PSUM slots (dead before bc=1).
            wsq_ps = psum.tile([1, SHARD_N], f32, tag="d", bufs=2, name="wsq_ps")
            for kc in range(4):
                sq = spool.tile([128, SHARD_N], bf16, tag="sq")
                nc.scalar.activation(sq[:], WT_b[:, kc, :], Act.Square)
                nc.tensor.matmul(
                    wsq_ps[0:1, 0:512], neg_half[:], sq[:, 0:512],
                    start=(kc == 0), stop=(kc == 3),
                )
                nc.tensor.matmul(
                    wsq_ps[0:1, 512:SHARD_N], neg_half[:], sq[:, 512:SHARD_N],
                    start=(kc == 0), stop=(kc == 3),
                )
            wrow = sb.tile([1, SHARD_N], bf16, tag="wrow")
            nc.scalar.copy(wrow[0:1, 0:512], wsq_ps[0:1, 0:512])
            nc.scalar.copy(wrow[0:1, 512:SHARD_N], wsq_ps[0:1, 512:SHARD_N])

            for bc in range(4):
                fsq_scr = spool.tile([128, H], bf16, tag="fsq_scr")
                fsq = cols.tile([128, 1], f32, tag="fsq")
                nc.scalar.activation(
                    fsq_scr[:], fR_b[:, bc, :], Act.Square, accum_out=fsq[:]
                )
                bias = cols.tile([128, 1], f32, tag="bias")
                nc.vector.tensor_scalar_mul(bias[:], fsq[:], -1.0 / SIGMA)

                d_ps = psum.tile([128, SHARD_N], f32, tag="d", bufs=2)
                for kc in range(4):
                    lhs = fT_b[:, kc, bc * 128:(bc + 1) * 128]
                    nc.tensor.matmul(
                        d_ps[:, 0:512], lhs, WT_b[:, kc, 0:512],
                        start=(kc == 0), stop=False,
                    )
                    nc.tensor.matmul(
                        d_ps[:, 512:SHARD_N], lhs, WT_b[:, kc, 512:SHARD_N],
                        start=(kc == 0), stop=False,
                    )
                nc.tensor.matmul(
                    d_ps[:, 0:512], ones_row[:], wrow[0:1, 0:512],
                    start=False, stop=True,
                )
                nc.tensor.matmul(
                    d_ps[:, 512:SHARD_N], ones_row[:], wrow[0:1, 512:SHARD_N],
                    start=False, stop=True,
                )
                e = dpool.tile([128, SHARD_N], f32, tag="e")
                nc.scalar.activation(
                    e[:, 0:512], d_ps[:, 0:512], Act.Exp,
                    bias=bias[:], scale=2.0 / SIGMA,
                )
                nc.scalar.activation(
                    e[:, 512:SHARD_N], d_ps[:, 512:SHARD_N], Act.Exp,
                    bias=bias[:], scale=2.0 / SIGMA,
                )
                dmax = dpool.tile([128, SHARD_C], f32, tag="dmax")
                nc.vector.reduce_max(
                    dmax[:],
                    e[:].rearrange("p (c f) -> p c f", f=NCEN),
                    axis=AX.X,
                )
                nc.sync.dma_start(dist[bc * 128:(bc + 1) * 128, :], dmax[:])

        with cmp.Else():
            # ----- Gram + stats branch (core 0) -----
            # W0 loads only here: distance cores never pay for the 4MB they
            # don't use.  Chunk-granular so the PE ha=0 pass streams.
            for kc in range(KC):
                nc.sync.dma_start(w_b[:, kc, :], W0_r[:, kc, :])
            # interleaved (ones | w_sq) stats pairs, one tile for all chunks
            stats64 = sb.tile([128, KC, 2], bf16, tag="stats64")
            nc.vector.memset(stats64[:, :, 0], 1.0)
            wsq_all = sb.tile([128, KC], f32, tag="wsq_all")
            for kc in range(KC):
                wsq_c = wsq_all[:, kc:kc + 1]
                if kc % 16 < 9:  # 18 on ACT, 14 on the GpSimd+DVE pipe
                    gsq = spool.tile([128, H], bf16, tag="gsq")
                    nc.scalar.activation(
                        gsq[:], w_b[:, kc, :], Act.Square, accum_out=wsq_c
                    )
                else:
                    dsq = spool.tile([128, H], bf16, tag="dsq")
                    nc.gpsimd.tensor_mul(dsq[:], w_b[:, kc, :], w_b[:, kc, :])
                    nc.vector.reduce_sum(wsq_c, dsq[:], axis=AX.X)
            nc.vector.tensor_copy(stats64[:, :, 1], wsq_all[:])

            g01 = psum.tile([128, 512], f32, tag="g01", name="g01")
            g1t = psum.tile([128, 384], f32, tag="g1t", name="g1t")
            g23 = psum.tile([128, 384], f32, tag="g23", name="g23")
            combo = psum.tile([128, 512], f32, tag="combo", name="combo_ps")
            # g blocks: ha=0 -> g01[:, 0:512]; ha=1 -> g1t[:, 0:384];
            # ha=2 -> g23[:, 0:256]; ha=3 -> g23[:, 256:384]
            g_ap = [g01[:, 0:512], g1t[:, 0:384], g23[:, 0:256], g23[:, 256:384]]
            for ha in range(4):
                for kc in range(KC):
                    nc.tensor.matmul(
                        g_ap[ha],
                        w_b[:, kc, ha * 128:(ha + 1) * 128],
                        w_b[:, kc, ha * 128:512],
                        start=(kc == 0), stop=(kc == KC - 1),
                    )
            # combo bank: rows 0-1 = [s; u] (h-major), row 2 = [A, B],
            # row 3 = cross-partition sum of the ||G||^2 accumulators
            for kc in range(KC):
                nc.tensor.matmul(
                    combo[0:2, :], stats64[:, kc, :], w_b[:, kc, :],
                    start=(kc == 0), stop=(kc == KC - 1),
                )

            # per-partition reductions.  DVE/ACT may read only ONE operand from
            # PSUM, so: ||G||^2 via ACT Square+accum (scale folds the 2x/4x
            # symmetry weights), su via an SBUF copy then DVE TTR.
            ttr_scr = spool.tile([128, 512], bf16, tag="ttr_scr")
            g_accs = []
            SQRT2 = 1.4142135623730951
            for ha in range(4):
                pieces = [(0, 128, SQRT2)]          # diag block: 2*x^2
                if ha < 3:
                    pieces.append((128, (4 - ha) * 128 - 128, 2.0))  # off: 4*x^2
                for off, width, sc in pieces:
                    acc = cols.tile([128, 1], f32, tag="gacc", name=f"gacc{ha}_{off}")
                    nc.scalar.activation(
                        ttr_scr[:, 0:width],
                        g_ap[ha][:, off:off + width],
                        Act.Square,
                        scale=sc,
                        accum_out=acc[:],
                    )
                    g_accs.append(acc)
            acc_big = cols.tile([128, 1], f32, tag="accbig")
            nc.vector.tensor_add(acc_big[:], g_accs[0][:], g_accs[1][:])
            for acc in g_accs[2:]:
                nc.vector.tensor_add(acc_big[:], acc_big[:], acc[:])

            # per-partition A,B partials on DVE, then one cross-partition
            # ones-matmul for [2||G||^2, A, B]
            ones_col = sb.tile([128, 1], f32, tag="ones_col")
            nc.vector.memset(ones_col[:], 1.0)
            cp = sb.tile([128, 3], f32, tag="cp")
            nc.vector.tensor_copy(cp[:, 0:1], acc_big[:])
            nc.vector.reduce_sum(cp[:, 1:2], wsq_all[:], axis=AX.X)
            wsq2 = sb.tile([128, KC], f32, tag="wsq2")
            nc.vector.tensor_mul(wsq2[:], wsq_all[:], wsq_all[:])
            nc.vector.reduce_sum(cp[:, 2:3], wsq2[:], axis=AX.X)
            nc.tensor.matmul(
                combo[64:65, 0:3], ones_col[:], cp[:], start=True, stop=True
            )

            # s,u rows to SBUF; put u on partition 0 next to s via DMA
            su_sb = sb.tile([2, 512], f32, tag="su_sb")
            nc.vector.tensor_copy(su_sb[:], combo[0:2, :])
            u_row = sb.tile([1, 512], f32, tag="u_row")
            nc.sync.dma_start(u_row[0:1, :], su_sb[1:2, :])
            ss_scr = sb.tile([1, 512], f32, tag="ss_scr")
            us_scr = sb.tile([1, 512], f32, tag="us_scr")
            nc.vector.tensor_mul(ss_scr[:], su_sb[0:1, :], su_sb[0:1, :])
            nc.vector.tensor_mul(us_scr[:], su_sb[0:1, :], u_row[:])
            scr = sb.tile([1, 16], f32, tag="scr")
            nc.vector.reduce_sum(scr[0:1, 11:12], ss_scr[:], axis=AX.X)
            nc.vector.reduce_sum(scr[0:1, 12:13], us_scr[:], axis=AX.X)

            # scalar assembly on partition 0: combo[64] = [2||G||^2, A, B]
            nc.vector.tensor_copy(scr[0:1, 10:11], combo[64:65, 0:1])
            nc.vector.tensor_copy(scr[0:1, 8:10], combo[64:65, 1:3])  # A, B
            t1 = scr[0:1, 0:1]
            mu = scr[0:1, 1:2]
            t2 = scr[0:1, 2:3]
            a2 = scr[0:1, 3:4]
            mu2 = scr[0:1, 4:5]
            rwv = scr[0:1, 5:6]
            t3 = scr[0:1, 6:7]
            A_ap = scr[0:1, 8:9]
            B_ap = scr[0:1, 9:10]
            g2_ap = scr[0:1, 10:11]   # 2*||G||^2
            ssq_ap = scr[0:1, 11:12]  # ||s||^2
            us_ap = scr[0:1, 12:13]   # u.s
            nc.vector.tensor_scalar_mul(t1, A_ap, float(MC))
            nc.vector.tensor_sub(t1, t1, ssq_ap)
            nc.vector.tensor_scalar_mul(mu, t1, INV_T)
            nc.vector.tensor_scalar_mul(t2, B_ap, float(MC))
            nc.vector.tensor_mul(a2, A_ap, A_ap)
            nc.vector.tensor_add(t2, t2, a2)
            nc.vector.tensor_add(t2, t2, g2_ap)
            nc.vector.tensor_scalar_mul(t3, us_ap, -4.0)
            nc.vector.tensor_add(t2, t2, t3)
            nc.vector.tensor_scalar_mul(t2, t2, INV_T)
            nc.vector.tensor_mul(mu2, mu, mu)
            nc.vector.tensor_sub(rwv, t2, mu2)
            nc.sync.dma_start(rw_out[0:1, 0:1], rwv)

    return nc


def _prep_inputs(f, W):
    bf = ml_dtypes.bfloat16
    f = np.ascontiguousarray(np.asarray(f, dtype=np.float32))
    w_flat = np.ascontiguousarray(np.asarray(W, dtype=np.float32).reshape(MC, H))

    Wb = w_flat.astype(bf)
    W0_full = np.zeros((MC_PAD, H), dtype=bf)
    W0_full[:MC] = Wb
    fT_np = np.ascontiguousarray(f.astype(bf).T)
    z_W0 = np.zeros((MC_PAD, H), dtype=bf)
    z_fT = np.zeros((H, B), dtype=bf)
    z_fR = np.zeros((B, H), dtype=np.float32)
    z_WT = np.zeros((H, SHARD_N), dtype=bf)

    in_maps = [{"W0": W0_full, "fT": z_fT, "fR": z_fR, "WT": z_WT}]
    for k in range(1, N_CORES):
        r0 = SHARD_N * (k - 1)
        r1 = min(r0 + SHARD_N, MC)
        WT_np = np.zeros((H, SHARD_N), dtype=bf)
        WT_np[:, : r1 - r0] = Wb[r0:r1].T
        in_maps.append({"W0": z_W0, "fT": fT_np, "fR": f, "WT": WT_np})
    return in_maps


def kernel(f, W, trace=False):
    from concourse.bass_utils import run_bass_kernel_spmd

    nc = _CACHE.get("nc")
    if nc is None:
        nc = _build()
        _CACHE["nc"] = nc

    in_maps = _prep_inputs(f, W)
    kwargs = {}
    if trace:
        kwargs["trace_cores"] = [0, 4]
    res = run_bass_kernel_spmd(
        nc, in_maps, core_ids=list(range(N_CORES)), trace=trace, **kwargs
    )
    _CACHE["last_result"] = res

    out = np.empty((B, NCLS + 1), dtype=np.float32)
    for k in range(1, N_CORES):
        c0 = SHARD_C * (k - 1)
        ncls = min(SHARD_C, NCLS - c0)
        if ncls <= 0:
            continue
        out[:, c0:c0 + ncls] = res.results[k]["dist"][:, :ncls]
    out[:, NCLS] = res.results[0]["rw"][0, 0]
    return out



# revision 17
# speedup vs baseline: 1.3392x; 1.3392x over previous
"""ClusteringAffinity (vq_codebook) Trainium2 kernel — 8 NeuronCores, SPMD.

Math: out[:, :1000] = max over 4 centers of exp(-||f_b - w_{c,j}||^2 / sigma);
out[:, 1000] = rw, a variance-style regularizer over all pairwise center
distances.  The mc x mc pairwise matrix is never formed: with
  A = sum_i ||w_i||^2, B = sum_i ||w_i||^4, s = sum_i w_i,
  u = sum_i ||w_i||^2 w_i, G = W^T W  (h x h Gram),
  T = (mc^2 - mc)/2,
  S1 = mc*A - ||s||^2,  mu = S1/T,
  rw = [mc*B + A^2 - 4 u.s]/T - mu^2   +   2||G||_F^2 / T.

rw is ADDITIVE in the two bracketed terms, so it is sum-sharded over two
cores and the partials are gathered and summed on the host (the same
gather-of-partials pattern the sharding hint suggests; an on-device 8-rank
collective costs ~80us on this stack, more than the whole kernel).

Sharding:
  core 0:     stats term  (bf16 W -> A, B; combo matmul -> s, u)
  core 1:     Gram term   (fp8e4 W at scale 64, DoubleRow matmul = 0.5 cyc/col)
  cores 2..7: distance for 167 classes each (668 centers, padded to 672), fp16.

Numerics (validated against fp64 in numpy): rw rel err ~7e-4 (fp8 Gram) +
~4e-4 (two-stage bf16 reduce for wsq); distance rel err ~5e-4 with fp16.
Tolerance is 2e-2.

Performance structure (from perfetto traces of the previous versions):
 - DMA descriptor GENERATION (~7ns/desc, serialized) dominated the original:
   all big loads now use row-permuted layouts ("(p kc) h") so each partition
   is one contiguous multi-KB descriptor (128 descs per dma_start).  The
   Gram/stats are invariant to mc-row permutation; fT/WT share one
   h->(p,kc) map so the contraction stays consistent.
 - Branch dispatch costs ~10us before in-branch DMAs can issue, so every
   load is issued pre-branch with cond=(pid==k); skipped DMAs only pay
   descriptor generation.
 - The distance term accumulates  wsq - 2 f.w  directly in PSUM (fT is
   host-scaled by -2), takes reduce_MIN over centers, then one small exp:
   max_j exp(-d_j) = exp(-min_j d_j).
 - Distance output is packed [128, 4*167] (one 128-desc DMA) and unsharded
   on the host.
"""

import numpy as np
import ml_dtypes
from contextlib import ExitStack

B = 512
H = 512
NCLS = 1000
NCEN = 4
SIGMA = 10.0
MC = 4000
MC_PAD = 4096
KC = MC_PAD // 128          # 32 contraction chunks
NKP = KC // 2               # 16 DoubleRow k-pairs
N_CORES = 8
D_CORES = 6                 # distance cores: 2..7
SHARD_C = 167               # classes per distance core (6*167 = 1002 >= 1000)
SHARD_N = SHARD_C * NCEN    # 668 centers
SHARD_NP = 672              # padded (PSUM splits 512 + 160)
FP8_SCALE = 64.0            # w*64 in fp8e4 keeps values out of subnormals
T_PAIRS = (MC * MC - MC) / 2.0
INV_T = 1.0 / T_PAIRS

_CACHE = {}


def _install_tile_patch():
    """walrus on this stack rejects >1 sync-wait on CTRL-class (Drain/NoOp)
    instructions; TileContext's tail drain carries one wait per active proc.
    Emit one SP nop per wait instead."""
    import re
    import concourse.tile as tile
    from bass_rust import ScopedClock, VectorClock

    if getattr(tile.TileContext, "_drain_split_patched", False):
        return

    def _clock_values(vc):
        m = re.search(r"\[([0-9, ]*)\]", repr(vc))
        s = m.group(1).strip()
        return [int(x) for x in s.split(",")] if s else []

    def _patched(self, tick_clock, wait_clock):
        nc = self.nc
        vals = _clock_values(tick_clock.global_clock)
        for i, v in enumerate(vals):
            if v > 0:
                chunk = [0] * len(vals)
                chunk[i] = v
                nop = nc.sync.nop(nofuse=True, hint="tail_wait")
                wait_clock.add_sem_waits(
                    nop.ins, ScopedClock({None: VectorClock(chunk)})
                )
        nc.sync.drain()
        nc.all_engine_barrier()
        assert self.sems is not None
        popped = nc._tile_sem_poison_stack.pop()
        assert popped is self._sem_poison
        nc.clear_and_free_semaphores(list(self.sems.allocated().values()))
        nc.all_engine_barrier()

    tile.TileContext._drain_and_barrier = _patched
    tile.TileContext._drain_split_patched = True


def _install_wait_split_patch():
    """This walrus build accepts at most ONE sync-wait per instruction.
    Rewrite the BIR before compile: hoist excess on_wait entries onto
    same-engine NoOps inserted immediately before the instruction."""
    import json
    import concourse.bass2jax as bass2jax
    import concourse.bass_utils as bass_utils

    if getattr(bass_utils, "_wait_split_patched", False):
        return
    orig = bass_utils.compile_bir_kernel

    # Opcodes with wide sem-update immediate fields; everything else is
    # capped at +1 on this walrus build.
    _WIDE_UPDATE = {"DMACopy", "EventSemaphore", "DMATranspose"}

    def _rewrite(bir_bytes):
        d = json.loads(bir_bytes)
        nid = 0
        changed = False
        for fn in d.get("functions", []):
            for blk in fn.get("blocks", []):
                insts = blk.get("instructions", [])
                new = []
                for inst in insts:
                    si = inst.get("sync_info")
                    waits = (si or {}).get("on_wait") or []
                    if len(waits) > 1:
                        changed = True
                        for w in waits[:-1]:
                            nid += 1
                            new.append({
                                "ins": [],
                                "name": f"WS-{nid}-{inst['name']}",
                                "opcode": "NoOp",
                                "outs": [],
                                "engine": inst["engine"],
                                "sync_info": {"on_update": [], "on_wait": [w]},
                                "text_hint": "wait_split",
                            })
                        si["on_wait"] = [waits[-1]]
                    new.append(inst)
                    ups = (si or {}).get("on_update") or []
                    if (
                        ups
                        and inst.get("opcode") not in _WIDE_UPDATE
                        and any(
                            u.get("update_mode") == "sem-add-imm"
                            and u.get("update_value", 0) > 1
                            for u in ups
                        )
                    ):
                        changed = True
                        keep, hoist = [], []
                        for u in ups:
                            if (
                                u.get("update_mode") == "sem-add-imm"
                                and u.get("update_value", 0) > 1
                            ):
                                hoist.append(u)
                            else:
                                keep.append(u)
                        si["on_update"] = keep
                        # Drain first: an EventSemaphore fires at engine
                        # commit-time, which for PE precedes the PSUM drain —
                        # signalling there would let consumers read stale PSUM.
                        nid += 1
                        new.append({
                            "debug": 0,
                            "ins": [],
                            "is_reset_sema": False,
                            "name": f"DR-{nid}-{inst['name']}",
                            "opcode": "Drain",
                            "outs": [],
                            "engine": inst["engine"],
                            "sync_info": {"on_update": [], "on_wait": []},
                        })
                        for u in hoist:
                            nid += 1
                            new.append({
                                "debug": 0,
                                "ins": [],
                                "name": f"US-{nid}-{inst['name']}",
                                "opcode": "EventSemaphore",
                                "outs": [],
                                "engine": inst["engine"],
                                "sync_info": {"on_update": [u], "on_wait": []},
                            })
                blk["instructions"] = new
        if not changed:
            return bir_bytes
        return json.dumps(d).encode()

    def patched(bir_json, tmpdir, neff_name="file.neff"):
        return orig(_rewrite(bir_json), tmpdir, neff_name=neff_name)

    bass_utils.compile_bir_kernel = patched
    bass2jax.compile_bir_kernel = patched
    bass_utils._wait_split_patched = True


def _build():
    import concourse.bass as bass
    import concourse.tile as tile
    from concourse import mybir

    _install_tile_patch()
    _install_wait_split_patch()

    dt = mybir.dt
    f32 = dt.float32
    bf16 = dt.bfloat16
    fp16 = dt.float16
    fp8 = dt.float8e4
    Act = mybir.ActivationFunctionType
    AX = mybir.AxisListType
    PM = mybir.MatmulPerfMode

    nc = bass.Bass()
    W16 = nc.dram_tensor("W16", [MC_PAD, H], bf16, kind="ExternalInput")
    W8 = nc.dram_tensor("W8", [MC_PAD, H], fp8, kind="ExternalInput")
    fT = nc.dram_tensor("fT", [H, B], fp16, kind="ExternalInput")   # = (-2f)^T
    WT = nc.dram_tensor("WT", [H, SHARD_NP], fp16, kind="ExternalInput")
    fR = nc.dram_tensor("fR", [B, H], f32, kind="ExternalInput")
    dist = nc.dram_tensor("dist", [128, NCEN * SHARD_C], f32, kind="ExternalOutput")
    rw_out = nc.dram_tensor("rw", [1, 1], f32, kind="ExternalOutput")

    with tile.TileContext(nc) as tc, ExitStack() as ctx:
        sb = ctx.enter_context(tc.tile_pool(name="sb", bufs=1))
        wpool = ctx.enter_context(tc.tile_pool(name="wpool", bufs=1))
        spool = ctx.enter_context(tc.tile_pool(name="spool", bufs=2))
        cols = ctx.enter_context(tc.tile_pool(name="cols", bufs=16))
        dpool = ctx.enter_context(tc.tile_pool(name="dpool", bufs=2))
        psum = ctx.enter_context(tc.tile_pool(name="psum", bufs=1, space="PSUM"))

        pid = nc.partition_id()
        is_stat = pid == 0
        is_gram = pid == 1
        is_dist = pid > 1

        # ---- all loads issue pre-branch, predicated per core: the branch
        # dispatch costs ~10us, the preamble only ~6; skipped DMAs just burn
        # descriptor generation.  Distance loads first (6 cores want them).
        fT_b = wpool.tile([128, 4, B], fp16, tag="ftb")      # h = p*4+kc
        WT_b = wpool.tile([128, 4, SHARD_NP], fp16, tag="wtb")
        fR_b = wpool.tile([128, 4, H], f32, tag="frb")
        w16 = wpool.tile([128, KC, H], bf16, tag="w16")      # mc = p*32+kc
        w8 = wpool.tile([128, KC, H], fp8, tag="w8")
        nc.sync.dma_start(
            WT_b[:], WT.rearrange("(p kc) n -> p kc n", p=128), cond=is_dist
        )
        nc.sync.dma_start(
            fT_b[:], fT.rearrange("(p kc) b -> p kc b", p=128), cond=is_dist
        )
        nc.sync.dma_start(
            fR_b[:], fR.rearrange("(bc p) h -> p bc h", p=128), cond=is_dist
        )
        # 2x2 identity for the PE transpose of [s; u]; shipped in core 0's
        # otherwise-unused fR slot (engine memsets can't start at partition 1)
        ident2 = sb.tile([2, 2], f32, tag="ident2")
        nc.sync.dma_start(ident2[:], fR[0:2, 0:2], cond=is_stat)
        # at most 8 register-offset (cond) DMAs fit per engine queue; the W
        # group loads go on the Activation engine's DGE queue instead.
        W16_r = W16.rearrange("(p kc) h -> p kc h", p=128)
        W8_r = W8.rearrange("(p kc) h -> p kc h", p=128)
        for g in range(4):
            sl = slice(g * 8, (g + 1) * 8)
            nc.scalar.dma_start(w16[:, sl, :], W16_r[:, sl, :], cond=is_stat)
            nc.scalar.dma_start(w8[:, sl, :], W8_r[:, sl, :], cond=is_gram)

        with tc.If(pid > 1) as cmp:
            # ===== distance branch (cores 2..7): 167 classes, fp16 =====
            ones_row = sb.tile([1, 128], fp16, tag="ones_row")
            nc.vector.memset(ones_row[:], 1.0)
            onec16 = sb.tile([128, 1], fp16, tag="onec16")
            nc.vector.memset(onec16[:], 1.0)

            # +||w||^2 as a [1, SHARD_NP] row (two PSUM pieces, aliased onto
            # gram-branch tags to stay within 8 PSUM banks)
            wsqA = psum.tile([1, 512], f32, tag="g1t", name="wsqA")
            wsqB = psum.tile([1, SHARD_NP - 512], f32, tag="combo", name="wsqB")
            for kc in range(4):
                sq = spool.tile([128, SHARD_NP], fp16, tag="sq")
                nc.scalar.activation(sq[:], WT_b[:, kc, :], Act.Square)
                nc.tensor.matmul(
                    wsqA[0:1, :], onec16[:], sq[:, 0:512],
                    start=(kc == 0), stop=(kc == 3),
                )
                nc.tensor.matmul(
                    wsqB[0:1, :], onec16[:], sq[:, 512:SHARD_NP],
                    start=(kc == 0), stop=(kc == 3),
                )
            wrow = sb.tile([1, SHARD_NP], fp16, tag="wrow")
            nc.scalar.copy(wrow[0:1, 0:512], wsqA[0:1, :])
            nc.scalar.copy(wrow[0:1, 512:SHARD_NP], wsqB[0:1, :])

            # biases: -||f_b||^2 / sigma, from fp32 f
            biases = []
            for bc in range(4):
                fsq_scr = spool.tile([128, H], bf16, tag="fsq_scr")
                fsq = cols.tile([128, 1], f32, tag="fsq")
                nc.scalar.activation(
                    fsq_scr[:], fR_b[:, bc, :], Act.Square, accum_out=fsq[:]
                )
                bias = cols.tile([128, 1], f32, tag="bias")
                nc.vector.tensor_scalar_mul(bias[:], fsq[:], -1.0 / SIGMA)
                biases.append(bias)

            # d_ps = wsq - 2 f.w  (fT is host-scaled by -2).  PE stream is
            # ordered so fw matmuls start before wsq/wrow are ready:
            # bc0 fw, bc1 fw, wrow0, wrow1, bc2 fw, wrow2, bc3 fw, wrow3.
            d_ps = [
                psum.tile([128, SHARD_NP], f32, tag="d", bufs=2, name=f"d_ps{i}")
                for i in range(4)
            ]

            def fw(bc):
                for kc in range(4):
                    lhs = fT_b[:, kc, bc * 128:(bc + 1) * 128]
                    nc.tensor.matmul(
                        d_ps[bc][:, 0:512], lhs, WT_b[:, kc, 0:512],
                        start=(kc == 0), stop=False,
                    )
                    nc.tensor.matmul(
                        d_ps[bc][:, 512:SHARD_NP], lhs, WT_b[:, kc, 512:SHARD_NP],
                        start=(kc == 0), stop=False,
                    )

            def wadd(bc):
                nc.tensor.matmul(
                    d_ps[bc][:, 0:512], ones_row[:], wrow[0:1, 0:512],
                    start=False, stop=True,
                )
                nc.tensor.matmul(
                    d_ps[bc][:, 512:SHARD_NP], ones_row[:], wrow[0:1, 512:SHARD_NP],
                    start=False, stop=True,
                )

            dmax = dpool.tile([128, 4, SHARD_C], f32, tag="dmax", bufs=1)

            def finish(bc):
                m = dpool.tile([128, SHARD_C], f32, tag="mins")
                nc.vector.tensor_reduce(
                    m[:],
                    d_ps[bc][:, 0:SHARD_N].rearrange("p (c f) -> p c f", f=NCEN),
                    axis=AX.X,
                    op=mybir.AluOpType.min,
                )
                # max_j exp(-d_j) = exp(-min_j d_j)
                nc.scalar.activation(
                    dmax[:, bc, :], m[:], Act.Exp,
                    bias=biases[bc][:], scale=-1.0 / SIGMA,
                )

            fw(0)
            fw(1)
            wadd(0)
            wadd(1)
            finish(0)
            fw(2)
            wadd(2)
            finish(1)
            fw(3)
            wadd(3)
            finish(2)
            finish(3)
            # one packed write: [p, bc, c] contiguous per partition, 128 descs
            nc.sync.dma_start(dist[:, :], dmax[:].rearrange("p bc c -> p (bc c)"))

        with cmp.Else():
            with tc.If(pid > 0) as cmp2:
                # ===== Gram branch (core 1): fp8 DoubleRow, rw_g = 2||G||^2/T
                g01 = psum.tile([128, 512], f32, tag="g01", name="g01")
                g1t = psum.tile([128, 384], f32, tag="g1t", name="g1t")
                g23 = psum.tile([128, 384], f32, tag="g23", name="g23")
                g_ap = [g01[:, 0:512], g1t[:, 0:384], g23[:, 0:256], g23[:, 256:384]]
                for ha in range(4):
                    for kp in range(NKP):
                        nc.tensor.matmul(
                            g_ap[ha],
                            w8[:, 2 * kp:2 * kp + 2, ha * 128:(ha + 1) * 128],
                            w8[:, 2 * kp:2 * kp + 2, ha * 128:512],
                            start=(kp == 0), stop=(kp == NKP - 1),
                            perf_mode=PM.DoubleRow,
                        )
                # ||G||^2 pieces: ACT Square+accum; scale folds 1/64^2 (fp8
                # scaling) and the 2x/4x triangle symmetry weights, so the
                # accumulated value is 2*||G||^2 directly.
                ttr_scr = spool.tile([128, 512], bf16, tag="ttr_scr")
                SC = 1.0 / (FP8_SCALE * FP8_SCALE)
                SQRT2 = 1.4142135623730951
                g_accs = []
                for ha in range(4):
                    pieces = [(0, 128, SQRT2 * SC)]
                    if ha < 3:
                        pieces.append((128, (4 - ha) * 128 - 128, 2.0 * SC))
                    for off, width, sc in pieces:
                        acc = cols.tile([128, 1], f32, tag="gacc", name=f"gacc{ha}_{off}")
                        nc.scalar.activation(
                            ttr_scr[:, 0:width],
                            g_ap[ha][:, off:off + width],
                            Act.Square,
                            scale=sc,
                            accum_out=acc[:],
                        )
                        g_accs.append(acc)
                acc_big = cols.tile([128, 1], f32, tag="accbig")
                nc.vector.tensor_add(acc_big[:], g_accs[0][:], g_accs[1][:])
                for acc in g_accs[2:]:
                    nc.vector.tensor_add(acc_big[:], acc_big[:], acc[:])
                ones_col = sb.tile([128, 1], f32, tag="ones_col")
                nc.vector.memset(ones_col[:], 1.0)
                gsc = psum.tile([1, 1], f32, tag="combo", name="gsc")
                nc.tensor.matmul(gsc[:], ones_col[:], acc_big[:], start=True, stop=True)
                rwg = sb.tile([1, 1], f32, tag="rwg")
                nc.vector.tensor_scalar_mul(rwg[:], gsc[:], INV_T)
                nc.sync.dma_start(rw_out[0:1, 0:1], rwg[:])

            with cmp2.Else():
                # ===== stats branch (core 0): A, B, s, u from bf16 W =====
                stats64 = sb.tile([128, KC, 2], bf16, tag="stats64")
                nc.vector.memset(stats64[:, :, 0], 1.0)
                wsq_all = sb.tile([128, KC], f32, tag="wsq_all")
                combo = psum.tile([128, 512], f32, tag="combo", name="combo_ps")
                for kc in range(KC):
                    wsq_c = wsq_all[:, kc:kc + 1]
                    r = kc % 32
                    # 17 chunks on ACT; 6 on DVE; 9 muls on GpSimd with the
                    # cheap two-stage (bf16 partials) reduce on DVE.
                    on_act = (r % 2 == 0) or r in (1, 31)
                    on_dve = (not on_act) and r in (3, 9, 15, 21, 27, 29)
                    if on_act:
                        gsq = spool.tile([128, H], bf16, tag="gsq")
                        nc.scalar.activation(
                            gsq[:], w16[:, kc, :], Act.Square, accum_out=wsq_c
                        )
                    else:
                        psq = spool.tile([128, H], bf16, tag="psq")
                        if on_dve:
                            nc.vector.tensor_mul(psq[:], w16[:, kc, :], w16[:, kc, :])
                        else:
                            nc.gpsimd.tensor_mul(psq[:], w16[:, kc, :], w16[:, kc, :])
                        red4 = cols.tile([128, 4], bf16, tag="red4")
                        # bf16 partials halve DVE reduce time (2x mode);
                        # validated: adds ~4e-4 rel on rw (tolerance 2e-2)
                        with nc.allow_low_precision(reason="bf16 partial sums"):
                            nc.vector.reduce_sum(
                                red4[:], psq[:].rearrange("p (s x) -> p s x", s=4),
                                axis=AX.X,
                            )
                        nc.vector.reduce_sum(wsq_c, red4[:], axis=AX.X)
                    nc.vector.tensor_copy(stats64[:, kc:kc + 1, 1], wsq_c)
                    nc.tensor.matmul(
                        combo[0:2, :], stats64[:, kc, :], w16[:, kc, :],
                        start=(kc == 0), stop=(kc == KC - 1),
                    )

                # per-partition partials: A, B, |s|^2, u.s -> one ones-matmul
                ones_col = sb.tile([128, 1], f32, tag="ones_col")
                nc.vector.memset(ones_col[:], 1.0)
                cp = sb.tile([128, 4], f32, tag="cp")
                nc.vector.reduce_sum(cp[:, 0:1], wsq_all[:], axis=AX.X)
                wsq2 = sb.tile([128, KC], f32, tag="wsq2")
                nc.vector.tensor_mul(wsq2[:], wsq_all[:], wsq_all[:])
                nc.vector.reduce_sum(cp[:, 1:2], wsq2[:], axis=AX.X)

                # s,u: [2,512] PSUM rows -> SBUF -> PE-transpose to columns
                su_sb = sb.tile([2, 512], f32, tag="su_sb")
                nc.scalar.copy(su_sb[:], combo[0:2, :])
                su_t = psum.tile([128, 8], f32, tag="g01", name="su_t")
                for hb in range(4):
                    nc.tensor.transpose(
                        su_t[:, 2 * hb:2 * hb + 2],
                        su_sb[:, hb * 128:(hb + 1) * 128],
                        ident2[:],
                    )
                su_s = sb.tile([128, 4, 2], f32, tag="su_s")
                nc.scalar.copy(su_s[:], su_t[:])
                ssp = sb.tile([128, 4, 2], f32, tag="ssp")
                nc.vector.tensor_mul(ssp[:, :, 0], su_s[:, :, 0], su_s[:, :, 0])
                nc.vector.tensor_mul(ssp[:, :, 1], su_s[:, :, 0], su_s[:, :, 1])
                nc.vector.reduce_sum(
                    cp[:, 2:4], ssp[:].rearrange("p s x -> p x s"), axis=AX.X
                )
                cpsc = psum.tile([1, 4], f32, tag="g1t", name="cpsc")
                nc.tensor.matmul(cpsc[:], ones_col[:], cp[:], start=True, stop=True)

                scr = sb.tile([1, 16], f32, tag="scr")
                nc.vector.tensor_copy(scr[0:1, 8:12], cpsc[0:1, :])
                t1 = scr[0:1, 0:1]
                mu = scr[0:1, 1:2]
                t2 = scr[0:1, 2:3]
                a2 = scr[0:1, 3:4]
                mu2 = scr[0:1, 4:5]
                rwv = scr[0:1, 5:6]
                t3 = scr[0:1, 6:7]
                A_ap = scr[0:1, 8:9]
                B_ap = scr[0:1, 9:10]
                ssq_ap = scr[0:1, 10:11]  # ||s||^2
                us_ap = scr[0:1, 11:12]   # u.s
                nc.vector.tensor_scalar_mul(t1, A_ap, float(MC))
                nc.vector.tensor_sub(t1, t1, ssq_ap)
                nc.vector.tensor_scalar_mul(mu, t1, INV_T)
                nc.vector.tensor_scalar_mul(t2, B_ap, float(MC))
                nc.vector.tensor_mul(a2, A_ap, A_ap)
                nc.vector.tensor_add(t2, t2, a2)
                nc.vector.tensor_scalar_mul(t3, us_ap, -4.0)
                nc.vector.tensor_add(t2, t2, t3)
                nc.vector.tensor_scalar_mul(t2, t2, INV_T)
                nc.vector.tensor_mul(mu2, mu, mu)
                nc.vector.tensor_sub(rwv, t2, mu2)
                nc.sync.dma_start(rw_out[0:1, 0:1], rwv)

    return nc


def _prep_inputs(f, W):
    bf = ml_dtypes.bfloat16
    f8 = ml_dtypes.float8_e4m3
    f = np.ascontiguousarray(np.asarray(f, dtype=np.float32))
    w_flat = np.ascontiguousarray(np.asarray(W, dtype=np.float32).reshape(MC, H))

    W16_full = np.zeros((MC_PAD, H), dtype=bf)
    W16_full[:MC] = w_flat.astype(bf)
    W8_full = np.zeros((MC_PAD, H), dtype=f8)
    W8_full[:MC] = (w_flat * FP8_SCALE).astype(f8)
    fT_np = np.ascontiguousarray((-2.0 * f).T.astype(np.float16))
    w16_t = w_flat.astype(np.float16).T  # [H, MC]

    z_W16 = np.zeros((MC_PAD, H), dtype=bf)
    z_W8 = np.zeros((MC_PAD, H), dtype=f8)
    z_fT = np.zeros((H, B), dtype=np.float16)
    z_fR = np.zeros((B, H), dtype=np.float32)
    z_WT = np.zeros((H, SHARD_NP), dtype=np.float16)

    fR_id = np.zeros((B, H), dtype=np.float32)
    fR_id[0, 0] = 1.0
    fR_id[1, 1] = 1.0
    in_maps = [
        {"W16": W16_full, "W8": z_W8, "fT": z_fT, "fR": fR_id, "WT": z_WT},
        {"W16": z_W16, "W8": W8_full, "fT": z_fT, "fR": z_fR, "WT": z_WT},
    ]
    for k in range(2, N_CORES):
        r0 = SHARD_N * (k - 2)
        r1 = min(r0 + SHARD_N, MC)
        WT_np = np.zeros((H, SHARD_NP), dtype=np.float16)
        WT_np[:, : r1 - r0] = w16_t[:, r0:r1]
        in_maps.append(
            {"W16": z_W16, "W8": z_W8, "fT": fT_np, "fR": f, "WT": WT_np}
        )
    return in_maps


def kernel(f, W, trace=False):
    from concourse.bass_utils import run_bass_kernel_spmd

    nc = _CACHE.get("nc")
    if nc is None:
        nc = _build()
        _CACHE["nc"] = nc

    in_maps = _prep_inputs(f, W)
    kwargs = {}
    if trace:
        kwargs["trace_cores"] = [0, 1, 4]
    res = run_bass_kernel_spmd(
        nc, in_maps, core_ids=list(range(N_CORES)), trace=trace, **kwargs
    )
    _CACHE["last_result"] = res

    out = np.empty((B, NCLS + 1), dtype=np.float32)
    for k in range(2, N_CORES):
        c0 = SHARD_C * (k - 2)
        ncls = min(SHARD_C, NCLS - c0)
        if ncls <= 0:
            continue
        # [128, 4*167] -> [4, 128, 167] -> rows bc*128+p
        d = res.results[k]["dist"].reshape(128, NCEN, SHARD_C).transpose(1, 0, 2)
        out[:, c0:c0 + ncls] = d.reshape(B, SHARD_C)[:, :ncls]
    # rw is sum-sharded: stats term (core 0) + Gram term (core 1)
    out[:, NCLS] = res.results[0]["rw"][0, 0] + res.results[1]["rw"][0, 0]
    return out
